# revision 17
# baseline (speedup 1.0000x reference)
"""Causal single-head attention (S=4096, dmodel=1024, dk=128) on 8 TRN2 cores.

Two-launch design:

Launch 1 (proj, bf16): core c loads X^T[:, 512c:512c+512] and all three W^T
packed in one bf16 tensor, computes unbiased K^T/V^T/Q^T slices (contiguous
rows), ships them bf16.  Biases are folded on the host: bk/bq are added to
K/Q before fp8 quantization, bv is added to the final output (softmax rows
sum to 1, so A@(V+bv) = A@V + bv).

Launch 2 (attention, fp8): core c handles interleaved Q rows c::8 against
full K/V.  Scores run as fp8 DoubleRow matmuls with a zero-padded second
k-tile (cost model: 0.5 cycles/output-col); the causal mask is a -600 bias
added into the score PSUM by the DVE before exp; exp runs on the scalar
engine writing fp8 tiles directly; AV and the denominator accumulate via
DoubleRow matmuls over key-chunk pairs.  Pair 0 (keys 0..255) runs bf16 to
keep early-row absmax error down.  Host divides ctx by den and adds bv.

Scheduling notes: small early PE warm-ups hold the p-state ramp (idle gaps
over ~1us reset it); a dummy activation preloads the exp table off the
critical path; score matmuls are issued 3 pairs ahead of the AV/sum matmuls
so the in-order PE queue never starves the scalar-engine exp stream; ctx/den
are copied out in column chunks as pairs finalize them; K/Q ship fp8 with
the bias fused into the DVE PSUM->SBUF copy.
"""

import math

import numpy as np
import ml_dtypes

S = 4096
D = 1024
DK = 128
NCORES = 8
SL = S // NCORES          # 512 q rows / core (interleaved c::8 in launch 2)
P = 128
NKC = S // P              # 32 key chunks
NPAIR = NKC // 2          # 16 key-chunk pairs
DCH = D // P              # 8 contraction chunks in launch 1

BF16 = ml_dtypes.bfloat16
F8 = ml_dtypes.float8_e4m3

_CACHE = {}

# launch-2 packed fp8 input column offsets
Q_OFF = 0                 # qT2 [128, 2*512]: t0 = Q^T (biased), t1 = zeros
K_OFF = 2 * SL            # K^T (biased) [128, 4096] + 128 pad cols
V_OFF = K_OFF + S + P     # vS pair layout [128, 16*2*128]
IN8_W = V_OFF + S


def _build_proj():
    import concourse.mybir as mybir
    from concourse import bacc
    from contextlib import ExitStack
    from concourse.tile import TileContext

    f32 = mybir.dt.float32
    bf16 = mybir.dt.bfloat16
    COPY = mybir.ActivationFunctionType.Copy

    fp8 = mybir.dt.float8e4
    CW = SL + 3 * DK      # packed row: x | wk | wv | wq
    nc = bacc.Bacc(None, name="proj")
    inw = nc.dram_tensor("inw", [D, CW], bf16, kind="ExternalInput")
    bias = nc.dram_tensor("bias", [DK, 2], f32, kind="ExternalInput")
    kqs = nc.dram_tensor("kqs", [P, 2 * SL], fp8, kind="ExternalOutput")
    vss = nc.dram_tensor("vss", [P, SL], bf16, kind="ExternalOutput")

    with TileContext(nc) as tc, ExitStack() as ctx:
        pool = ctx.enter_context(tc.tile_pool(name="pool", bufs=1))
        psum = ctx.enter_context(tc.tile_pool(name="psum", bufs=1, space="PSUM"))

        ain = pool.tile([P, DCH, CW], bf16)
        ain_r = inw[:, :].rearrange("(d p) w -> p d w", p=P)
        for d in range(DCH):
            nc.sync.dma_start(ain[:, d], ain_r[:, d])
        b_sb = pool.tile([DK, 2], f32)
        nc.sync.dma_start(b_sb, bias[:, :])

        psK = psum.tile([P, SL], f32, tag="k")
        psV = psum.tile([P, SL], f32, tag="v")
        psQ = psum.tile([P, SL], f32, tag="q")

        # warm-ups: keep the PE p-state ramp alive until the first X chunk
        # lands (~3.5us); psK is cleared by K0's start=True
        wu = pool.tile([P, 256], bf16)
        nc.vector.memset(wu, 1.0)
        for _ in range(8):
            nc.tensor.matmul(psK[0:1, 0:256], lhsT=wu[:, 0:1], rhs=wu,
                             start=True, stop=True, skip_group_check=True)

        xs = ain[:, :, 0:SL]
        for d in range(DCH):
            st, sp = d == 0, d == DCH - 1
            nc.tensor.matmul(psV, lhsT=ain[:, d, SL + DK:SL + 2 * DK],
                             rhs=xs[:, d], start=st, stop=sp)
            nc.tensor.matmul(psK, lhsT=ain[:, d, SL:SL + DK], rhs=xs[:, d],
                             start=st, stop=sp)
            nc.tensor.matmul(psQ, lhsT=ain[:, d, SL + 2 * DK:SL + 3 * DK],
                             rhs=xs[:, d], start=st, stop=sp)

        # V finishes first (bf16, biggest transfer) -> ACT copy + first DMA;
        # K on DVE; Q split across both engines so its copy lands early
        v_sb = pool.tile([P, SL], bf16)
        nc.scalar.activation(v_sb, psV, COPY)
        nc.scalar.dma_start(vss[:, :], v_sb)
        kq_sb = pool.tile([P, 2 * SL], fp8)
        nc.vector.tensor_scalar_add(kq_sb[:, 0:SL], psK, b_sb[:, 0:1])
        nc.vector.tensor_scalar_add(kq_sb[:, SL:], psQ, b_sb[:, 1:2])
        nc.sync.dma_start(kqs[:, :], kq_sb)

    nc.finalize()
    return nc


def _build_attn():
    import concourse.mybir as mybir
    from concourse import bacc
    from contextlib import ExitStack
    from concourse.tile import TileContext

    f32 = mybir.dt.float32
    bf16 = mybir.dt.bfloat16
    fp8 = mybir.dt.float8e4
    DR = mybir.MatmulPerfMode.DoubleRow
    EXP = mybir.ActivationFunctionType.Exp
    SCALE = 1.0 / math.sqrt(DK)

    nc = bacc.Bacc(None, name="attn")
    in8 = nc.dram_tensor("in8", [P, IN8_W], fp8, kind="ExternalInput")
    v0mb = nc.dram_tensor("v0mb", [P, 2 * P + 64], bf16, kind="ExternalInput")
    ctxd = nc.dram_tensor("ctxd", [DK, SL], f32, kind="ExternalOutput")
    dend = nc.dram_tensor("dend", [1, SL], f32, kind="ExternalOutput")

    with TileContext(nc) as tc, ExitStack() as ctx:
        pool = ctx.enter_context(tc.tile_pool(name="pool", bufs=1))
        epool = ctx.enter_context(tc.tile_pool(name="epool", bufs=3))
        pscore = ctx.enter_context(tc.tile_pool(name="pscore", bufs=3,
                                                space="PSUM"))
        pacc = ctx.enter_context(tc.tile_pool(name="pacc", bufs=1,
                                              space="PSUM"))

        sb8 = pool.tile([P, IN8_W], fp8)
        v0 = pool.tile([P, 2 * P + 64], bf16)

        # preload the exp activation table off the critical path
        edum = pool.tile([P, 8], bf16)
        nc.vector.memset(edum, 0.0)
        nc.scalar.activation(edum, edum, EXP, scale=SCALE)

        # DMA pieces ordered so each pair's K/V arrives just ahead of use.
        # Pair n's scores read K chunks [2n, 2n+3) (the zero-padded t=1 view
        # spills one chunk), so K cuts sit on odd chunk boundaries.
        pieces = [(0, K_OFF + 3 * P),               # qT2 + K chunks 0-2
                  (K_OFF + 3 * P, K_OFF + 9 * P),   # K chunks 3-8
                  (V_OFF, V_OFF + 4 * 2 * P),       # vS pairs 1-4
                  (K_OFF + 9 * P, K_OFF + 25 * P),  # K chunks 9-24
                  (K_OFF + 25 * P, K_OFF + S + P),  # K chunks 25-31 + pad
                  (V_OFF + 4 * 2 * P, IN8_W)]       # vS pairs 5-15
        nc.sync.dma_start(sb8[:, 0:K_OFF + 3 * P], in8[:, 0:K_OFF + 3 * P])
        nc.sync.dma_start(v0, v0mb[:, :])
        for lo, hi in pieces[1:]:
            nc.sync.dma_start(sb8[:, lo:hi], in8[:, lo:hi])

        qt2 = sb8[:, Q_OFF:Q_OFF + 2 * SL].rearrange("p (t n) -> p t n", t=2)
        mb = v0[:, 2 * P:].rearrange("p (t j) -> p t j", t=2)

        ones8 = pool.tile([P, 2, 32], fp8)
        nc.vector.memset(ones8, 1.0)
        ones_bf = pool.tile([P, 32], bf16)
        nc.vector.memset(ones_bf, 1.0)

        psAV = pacc.tile([DK, SL], f32, tag="av")
        psSum = pacc.tile([32, SL], f32, tag="sum")

        # warm-ups: hold the PE ramp until piece A lands (~2.8us); cleared by
        # pair 0's start=True
        wu = pool.tile([P, 256], bf16)
        nc.vector.memset(wu, 1.0)
        for _ in range(5):
            nc.tensor.matmul(psSum[0:1, 0:256], lhsT=wu[:, 0:1], rhs=wu,
                             start=True, stop=True, skip_group_check=True)

        e0 = pool.tile([P, 2, SL], bf16)
        ctx_sb = pool.tile([DK, SL], f32)
        den_sb = pool.tile([1, SL], f32)

        AHEAD = 3
        psS_t = {}

        def scores(n):
            NW = SL - 32 * n
            qlo = 32 * n
            psS = pscore.tile([P, 2, SL], f32, tag="s")
            psS_t[n] = psS
            for t in range(2):
                m = 2 * n + t
                lhsT = sb8[:, K_OFF + P * m:K_OFF + P * m + 2 * P].rearrange(
                    "p (t m2) -> p t m2", t=2)
                nc.tensor.matmul(psS[:, t, :NW], lhsT=lhsT,
                                 rhs=qt2[:, :, qlo:qlo + NW],
                                 start=True, stop=True, perf_mode=DR,
                                 skip_group_check=True)
            nc.vector.tensor_tensor(psS[:, :, 0:32], psS[:, :, 0:32], mb,
                                    mybir.AluOpType.add)

        for n in range(AHEAD):
            scores(n)

        for n in range(NPAIR):
            NW = SL - 32 * n
            qlo = 32 * n
            psS = psS_t.pop(n)
            last = n == NPAIR - 1
            if n + AHEAD < NPAIR:
                scores(n + AHEAD)
            if n == 0:
                nc.scalar.activation(e0[:, :, :NW], psS[:, :, :NW], EXP,
                                     scale=SCALE)
                for t in range(2):
                    nc.tensor.matmul(psAV, lhsT=v0[:, P * t:P * (t + 1)],
                                     rhs=e0[:, t, :], start=(t == 0),
                                     stop=False, skip_group_check=True)
                    nc.tensor.matmul(psSum, lhsT=ones_bf, rhs=e0[:, t, :],
                                     start=(t == 0), stop=False,
                                     skip_group_check=True)
            else:
                e4 = epool.tile([P, 2, SL], fp8, tag="e")
                nc.scalar.activation(e4[:, :, :NW], psS[:, :, :NW], EXP,
                                     scale=SCALE)
                vS = sb8[:, V_OFF + 2 * P * n:V_OFF + 2 * P * (n + 1)
                         ].rearrange("p (t m) -> p t m", t=2)
                nc.tensor.matmul(psAV[:, qlo:], lhsT=vS, rhs=e4[:, :, :NW],
                                 start=False, stop=last, perf_mode=DR,
                                 skip_group_check=True)
                nc.tensor.matmul(psSum[:, qlo:], lhsT=ones8,
                                 rhs=e4[:, :, :NW], start=False, stop=last,
                                 perf_mode=DR, skip_group_check=True)
            # ship finalized ctx/den column chunks while the exp stream runs
            # (n>=13 so the copies never queue ahead of a pending mask-add on
            # the in-order DVE)
            if n == 7:
                nc.vector.tensor_copy(ctx_sb[:, 0:256], psAV[:, 0:256])
                nc.sync.dma_start(ctxd[:, 0:256], ctx_sb[:, 0:256])
            elif n == 13:
                nc.vector.tensor_copy(ctx_sb[:, 256:384], psAV[:, 256:384])
                nc.sync.dma_start(ctxd[:, 256:384], ctx_sb[:, 256:384])
            elif n == 14:
                nc.vector.tensor_copy(den_sb[:, 0:480], psSum[0:1, 0:480])
                nc.vector.tensor_copy(ctx_sb[:, 384:480], psAV[:, 384:480])
                nc.sync.dma_start(ctxd[:, 384:480], ctx_sb[:, 384:480])

        # ACT is idle after the last exp: final ctx piece there, in parallel
        # with the DVE den tail; den ships first (tiny transfer)
        nc.scalar.activation(ctx_sb[:, 480:], psAV[:, 480:],
                             mybir.ActivationFunctionType.Copy)
        nc.vector.tensor_copy(den_sb[:, 480:], psSum[0:1, 480:])
        nc.sync.dma_start(dend[:, :], den_sb)
        nc.scalar.dma_start(ctxd[:, 480:], ctx_sb[:, 480:])

    nc.finalize()
    return nc


def get_ncs():
    if "nc1" not in _CACHE:
        _CACHE["nc1"] = _build_proj()
        _CACHE["nc2"] = _build_attn()
    return _CACHE["nc1"], _CACHE["nc2"]


def make_in_maps1(inputs):
    X = np.asarray(inputs["X"], np.float32)
    XT16 = np.ascontiguousarray(X.T).astype(BF16)
    WkT = np.ascontiguousarray(np.asarray(inputs["Wk"], np.float32).T
                               ).astype(BF16)
    WvT = np.ascontiguousarray(np.asarray(inputs["Wv"], np.float32).T
                               ).astype(BF16)
    WqT = np.ascontiguousarray(np.asarray(inputs["Wq"], np.float32).T
                               ).astype(BF16)
    bias = np.stack([np.asarray(inputs["bk"], np.float32),
                     np.asarray(inputs["bq"], np.float32)], axis=1)
    maps = []
    for c in range(NCORES):
        xst = XT16[:, SL * c:SL * (c + 1)]
        inw = np.ascontiguousarray(
            np.concatenate([xst, WkT, WvT, WqT], axis=1))
        maps.append({"inw": inw, "bias": bias})
    return maps


def make_in_maps2(res1, inputs):
    ktf = np.concatenate([np.asarray(r["kqs"])[:, 0:SL] for r in res1],
                         axis=1)                    # [128, 4096] K^T+bk fp8
    qtf = np.concatenate([np.asarray(r["kqs"])[:, SL:] for r in res1],
                         axis=1)                    # [128, 4096] Q^T+bq fp8
    vtf = np.concatenate([np.asarray(r["vss"]) for r in res1],
                         axis=1)                    # [128, 4096] V^T bf16
    kt8 = np.zeros((P, S + P), F8)
    kt8[:, :S] = ktf.astype(F8)
    vrows8 = np.ascontiguousarray(vtf.T).astype(np.float32).astype(F8)
    vS = vrows8.reshape(NPAIR, 2, P, DK).transpose(2, 0, 1, 3).reshape(P, S)
    v0 = np.ascontiguousarray(
        vtf[:, 0:2 * P].T.reshape(2, P, DK).transpose(1, 0, 2).reshape(P, 2 * DK))

    kr = np.arange(P)[:, None]
    jj = np.arange(16)[None, :]
    maps = []
    for c in range(NCORES):
        in8 = np.zeros((P, IN8_W), F8)
        in8[:, 0:SL] = qtf[:, c::NCORES].astype(F8)
        in8[:, K_OFF:K_OFF + S + P] = kt8
        in8[:, V_OFF:] = vS
        # mask bias template [128, 2, 32]: chunk 2n cols j'<16 masked where
        # key p > 8j'+c; chunk 2n+1: cols j'<16 all masked, cols 16..31
        # masked where p > 8(j'-16)+c
        mb = np.zeros((P, 2, 32), np.float32)
        mb[:, 0, 0:16] = np.where(kr > 8 * jj + c, -600.0, 0.0)
        mb[:, 1, 0:16] = -600.0
        mb[:, 1, 16:32] = np.where(kr > 8 * jj + c, -600.0, 0.0)
        v0mb = np.concatenate([v0, mb.reshape(P, 64).astype(BF16)],
                              axis=1).astype(BF16)
        maps.append({"in8": in8, "v0mb": np.ascontiguousarray(v0mb)})
    return maps


LAST_RESULTS = None


def kernel(**inputs) -> np.ndarray:
    global LAST_RESULTS
    from concourse.bass_utils import run_bass_kernel_spmd

    nc1, nc2 = get_ncs()
    res1 = run_bass_kernel_spmd(nc1, make_in_maps1(inputs),
                                core_ids=list(range(NCORES)))
    res2 = run_bass_kernel_spmd(nc2, make_in_maps2(res1.results, inputs),
                                core_ids=list(range(NCORES)))
    LAST_RESULTS = (res1, res2)
    bv = np.asarray(inputs["bv"], np.float32)
    out = np.empty((S, DK), np.float32)
    for c in range(NCORES):
        ctx = np.asarray(res2.results[c]["ctxd"], np.float32)
        den = np.asarray(res2.results[c]["dend"], np.float32)
        out[c::NCORES] = (ctx / den).T + bv
    return out
